# revision 26
# baseline (speedup 1.0000x reference)
"""CSPN (7x7 per-pixel spatial propagation) Trainium2 kernel.

Problem: out[b,0,y,x] = sum_{i,j in 0..6} gw[b, 7i+j, y+3, x+3] * src(y+3-i, x+3-j)
where src = hn (zero-padded outside [0,512)) except the center tap (i=j=3)
which uses h0. Shapes: gw [8,49,518,518] f32, hn/h0 [8,1,512,512] f32.

Strategy: pure data parallel - one batch element per NeuronCore (8 cores).

Layout: image row r lives at partition r//4, row-block b = r%4 (each
partition holds 4 consecutive rows). A vertical shift of up to +-3 rows
then moves at most +-1 partition, and every shifted read window is a
plain strided slice of ONE halo tensor:

    s0[p, m, 3+c] = hn[4p + m - 3, c]   for m in [0, 10)

Tap (i, j) with dr = 3-i, dc = 3-j reads s0[:, dr+3 : dr+7, 3+dc : 515+dc].
The slice's element offset is even exactly when j is even; a second copy
s1 one slot to the right serves odd-j taps so bf16 DVE reads stay
4B-aligned (2x mode).

The halo is built with a single 1 MB DMA of hn: the mid blocks (m=3..6)
are cast from the f32 staging tile, and the +-1-partition-shifted blocks
(m=0..2, 7..9) are produced on the otherwise-idle PE array as matmuls
with sub/super-diagonal permutation matrices (built via affine_select),
whose all-zero edge columns also provide the top/bottom zero padding for
free. After startup the DMA engines stream nothing but the 49
guide-weight planes (51.4 MB/core - the memory-roofline term), h0, and
the final 1 MB output store; the modeled DMA bus never idles mid-stream.

Engine roles:
 - SP sequencer: pure DMA issue ring (staging, h0, every weight plane,
   output stores) - no compute waits can stall it.
 - ACT: all f32->bf16 weight casts (so DVE multiplies run in 2x mode).
 - DVE: per-tap multiply + accumulate chain (bf16).
 - GpSimd (Pool): takes six early taps on a second accumulator (merged
   once mid-stream) plus the first tail tap per block, keeping DVE
   slack so the final adds fire as soon as the last weight bytes land.
 - PE: the six halo shift matmuls.

Tap order runs the shift-free row (i=3) first and the i=0 row last, so
the PE-produced halo blocks are needed only well after they are ready.
Tail: the last taps shrink to half-tiles then per-row-block quarters;
the final tap's quarters multiply straight from f32 weights (no cast
hop) and the final add emits f32 into outf, which SP streams out.
"""

import numpy as np

_CACHE = {}

# Row i=3 (no vertical shift) first, i=0 (needs all up-blocks) last.
TAP_ORDER = [7 * i + j for i in (3, 4, 2, 5, 1, 6, 0) for j in range(7)]
POOL_POS = (1, 4, 7, 10, 13, 16)  # positions offloaded to GpSimd
MERGE_POS = 28  # position after which acc2 merges into acc (must be < 30)


def _build_nc():
    import concourse.bacc as bacc
    import concourse.mybir as mybir
    import concourse.tile as tile

    F32 = mybir.dt.float32
    BF16 = mybir.dt.bfloat16
    MULT = mybir.AluOpType.mult
    ADD = mybir.AluOpType.add
    EQ = mybir.AluOpType.is_equal

    nc = bacc.Bacc("TRN2", target_bir_lowering=False, debug=False, num_devices=8)
    gw = nc.dram_tensor("gw", [49, 518, 518], F32, kind="ExternalInput").ap()
    hn = nc.dram_tensor("hn", [512, 512], F32, kind="ExternalInput").ap()
    h0 = nc.dram_tensor("h0", [512, 512], F32, kind="ExternalInput").ap()
    out = nc.dram_tensor("out", [512, 512], F32, kind="ExternalOutput").ap()

    with tile.TileContext(nc) as tc:
        with (
            tc.tile_pool(name="persist", bufs=1) as pp,
            tc.tile_pool(name="wf", bufs=4) as wfp,
            tc.tile_pool(name="wb", bufs=4) as wbp,
            tc.tile_pool(name="wb2", bufs=2) as wb2p,
            tc.tile_pool(name="prod", bufs=2) as prp,
            tc.tile_pool(name="p2", bufs=2) as p2p,
            tc.tile_pool(name="wq12", bufs=12) as wqp,
            tc.tile_pool(name="whalf", bufs=4) as whp,
            tc.tile_pool(name="wsmall", bufs=6) as wsp,
            tc.tile_pool(name="prodq", bufs=4) as pqp,
            tc.tile_pool(name="ps", bufs=2, space="PSUM") as psp,
        ):
            # --- halo staging -------------------------------------------
            stage = pp.tile([128, 4, 512], F32, tag="stage")
            nc.sync.dma_start(out=stage[:], in_=hn.rearrange("(p b) x -> p b x", b=4))
            h0f = pp.tile([128, 4, 512], F32)
            nc.sync.dma_start(out=h0f[:], in_=h0.rearrange("(p b) x -> p b x", b=4))

            s0 = pp.tile([128, 10, 520], BF16, tag="s0")
            s1 = pp.tile([128, 10, 520], BF16, tag="s1")
            nc.vector.memset(s0[:, :, 0:3], 0.0)
            nc.vector.memset(s0[:, :, 515:520], 0.0)
            nc.vector.memset(s1[:, :, 0:4], 0.0)
            nc.vector.memset(s1[:, :, 516:520], 0.0)
            nc.scalar.copy(out=s0[:, 3:7, 3:515], in_=stage[:])
            nc.vector.tensor_copy(s1[:, 3:7, 4:516], s0[:, 3:7, 3:515])
            h0b = pp.tile([128, 4, 512], BF16)
            nc.scalar.copy(out=h0b[:], in_=h0f[:])

            # Shift matrices: Tup[q, p] = [q == p+1], Tdn[q, p] = [q == p-1].
            # As matmul lhsT they realize out[p] = in[p+-1]; their all-zero
            # first/last columns zero the out-of-image rows automatically.
            ones = pp.tile([128, 128], BF16, tag="ones")
            nc.gpsimd.memset(ones[:], 1.0)
            tup = pp.tile([128, 128], BF16, tag="tup")
            nc.gpsimd.affine_select(
                out=tup[:], in_=ones[:], pattern=[[-1, 128]], compare_op=EQ,
                fill=0.0, base=-1, channel_multiplier=1,
            )
            tdn = pp.tile([128, 128], BF16, tag="tdn")
            nc.gpsimd.affine_select(
                out=tdn[:], in_=ones[:], pattern=[[-1, 128]], compare_op=EQ,
                fill=0.0, base=1, channel_multiplier=1,
            )

            # Up blocks m=7+r hold row 4p+4+r = mid block r of partition p+1;
            # dn blocks m=r hold row 4p+r-3 = mid block r+1 of partition p-1.
            # rhs reads the 4B-aligned s1 mid copy. Emission order matches
            # first use: i=4 needs m=2 first, i=2 needs m=7, etc.
            for kind, r in (("dn", 2), ("up", 0), ("dn", 1), ("up", 1), ("dn", 0), ("up", 2)):
                ps = psp.tile([128, 512], F32, tag="ps")
                if kind == "up":
                    nc.tensor.matmul(ps[:], tup[:], s1[:, 3 + r, 4:516])
                    dst = 7 + r
                else:
                    nc.tensor.matmul(ps[:], tdn[:], s1[:, 4 + r, 4:516])
                    dst = r
                nc.scalar.copy(out=s0[:, dst, 3:515], in_=ps[:])
            nc.vector.tensor_copy(s1[:, 0:3, 4:516], s0[:, 0:3, 3:515])
            nc.vector.tensor_copy(s1[:, 7:10, 4:516], s0[:, 7:10, 3:515])

            # --- tap machinery ------------------------------------------
            acc = pp.tile([128, 4, 512], BF16)
            acc2 = pp.tile([128, 4, 512], BF16)
            outf = pp.tile([128, 4, 512], F32)
            out_r = out.rearrange("(p b) x -> p b x", b=4)
            gw_r = [
                gw[t, 3:515, 3:515].rearrange("(p b) x -> p b x", b=4)
                for t in range(49)
            ]

            def src_for(t, b0=0, nb=4):
                i, j = t // 7, t % 7
                if t == 24:
                    return h0b[:, b0 : b0 + nb, :]
                dr, dc = 3 - i, 3 - j
                if j % 2 == 0:
                    return s0[:, dr + 3 + b0 : dr + 3 + b0 + nb, 3 + dc : 515 + dc]
                return s1[:, dr + 3 + b0 : dr + 3 + b0 + nb, 4 + dc : 516 + dc]

            # Positions 0..43 stream full-tile; six of them accumulate on
            # GpSimd into acc2, merged into acc once after the halves.
            pool_started = False
            for pos in range(36):
                t = TAP_ORDER[pos]
                wf = wfp.tile([128, 4, 512], F32, tag="wf")
                nc.sync.dma_start(out=wf[:], in_=gw_r[t])
                if pos in POOL_POS:
                    wb = wb2p.tile([128, 4, 512], BF16, tag="wb2")
                    nc.scalar.copy(out=wb[:], in_=wf[:])
                    if not pool_started:
                        nc.gpsimd.tensor_tensor(
                            out=acc2[:], in0=wb[:], in1=src_for(t), op=MULT
                        )
                        pool_started = True
                    else:
                        p2 = p2p.tile([128, 4, 512], BF16, tag="p2")
                        nc.gpsimd.tensor_tensor(
                            out=p2[:], in0=wb[:], in1=src_for(t), op=MULT
                        )
                        nc.gpsimd.tensor_tensor(
                            out=acc2[:], in0=acc2[:], in1=p2[:], op=ADD
                        )
                else:
                    wb = wbp.tile([128, 4, 512], BF16, tag="wb")
                    nc.scalar.copy(out=wb[:], in_=wf[:])
                    if pos == 0:
                        nc.vector.tensor_tensor(
                            out=acc[:], in0=wb[:], in1=src_for(t), op=MULT
                        )
                    else:
                        prod = prp.tile([128, 4, 512], BF16, tag="prod")
                        nc.vector.tensor_tensor(
                            out=prod[:], in0=wb[:], in1=src_for(t), op=MULT
                        )
                        nc.vector.tensor_tensor(
                            out=acc[:], in0=acc[:], in1=prod[:], op=ADD
                        )
                if pos == MERGE_POS:
                    nc.vector.tensor_tensor(out=acc[:], in0=acc[:], in1=acc2[:], op=ADD)

            # --- tail ----------------------------------------------------
            # The kernel's end waits on the cast->mult->add chain behind the
            # LAST weight bytes, so taper the pieces as the stream drains:
            # positions 36..46 run as half-tiles and 47..48 as per-row-block
            # quarters. The final tap's add emits f32 straight into outf
            # (no output cast), and SP streams the four block stores out.
            for pos in (36, 37, 38, 39, 40, 41, 42, 43, 44, 45, 46):
                t = TAP_ORDER[pos]
                for h in (0, 1):
                    wfh = whp.tile([128, 2, 512], F32, tag="wfh")
                    nc.sync.dma_start(out=wfh[:], in_=gw_r[t][:, 2 * h : 2 * h + 2, :])
                    wbh = wsp.tile([128, 2, 512], BF16, tag="wbh")
                    nc.scalar.copy(out=wbh[:], in_=wfh[:])
                    prodh = prp.tile([128, 2, 512], BF16, tag="prodh")
                    nc.vector.tensor_tensor(
                        out=prodh[:], in0=wbh[:], in1=src_for(t, 2 * h, 2), op=MULT
                    )
                    nc.vector.tensor_tensor(
                        out=acc[:, 2 * h : 2 * h + 2, :],
                        in0=acc[:, 2 * h : 2 * h + 2, :],
                        in1=prodh[:],
                        op=ADD,
                    )
            for pos in (42, 43):
                t = TAP_ORDER[pos]
                for b in range(4):
                    wq = wqp.tile([128, 512], F32, tag="wq")
                    nc.sync.dma_start(out=wq[:], in_=gw_r[t][:, b, :])
                    wbq = wsp.tile([128, 512], BF16, tag="wbq")
                    nc.scalar.copy(out=wbq[:], in_=wq[:])
                    pq = pqp.tile([128, 512], BF16, tag="prodq")
                    nc.vector.tensor_tensor(
                        out=pq[:], in0=wbq[:], in1=src_for(t, b, 1)[:, 0, :], op=MULT
                    )
                    nc.vector.tensor_tensor(
                        out=acc[:, b, :], in0=acc[:, b, :], in1=pq[:], op=ADD
                    )
            # Taps at positions 44/45: DVE does only the multiplies; the
            # otherwise-idle GpSimd engine pair-adds the two products so DVE
            # spends one add instead of two per block. (z = p44 + p45 on
            # Pool; acc += z on DVE.)
            pr44, pr45 = [], []
            for pos, keep in ((44, pr44), (45, pr45)):
                t = TAP_ORDER[pos]
                for b in range(4):
                    wq = wqp.tile([128, 512], F32, tag="wq")
                    nc.sync.dma_start(out=wq[:], in_=gw_r[t][:, b, :])
                    wbq = wsp.tile([128, 512], BF16, tag="wbq")
                    nc.scalar.copy(out=wbq[:], in_=wq[:])
                    pq = tkp.tile([128, 512], BF16, tag=f"p{pos}")
                    nc.vector.tensor_tensor(
                        out=pq[:], in0=wbq[:], in1=src_for(t, b, 1)[:, 0, :], op=MULT
                    )
                    keep.append(pq)
            zs = []
            for b in range(4):
                z = tkp.tile([128, 512], BF16, tag="z1")
                nc.gpsimd.tensor_tensor(
                    out=z[:], in0=pr44[b][:], in1=pr45[b][:], op=ADD
                )
                zs.append(z)
            for b in range(4):
                nc.vector.tensor_tensor(
                    out=acc[:, b, :], in0=acc[:, b, :], in1=zs[b][:], op=ADD
                )
            for pos in (46, 47, 48):
                t = TAP_ORDER[pos]
                last = pos == 48
                for b in range(4):
                    wq = wqp.tile([128, 512], F32, tag="wq")
                    nc.sync.dma_start(out=wq[:], in_=gw_r[t][:, b, :])
                    wbq = wsp.tile([128, 512], BF16, tag="wbq")
                    nc.scalar.copy(out=wbq[:], in_=wq[:])
                    pq = pqp.tile([128, 512], BF16, tag="prodq")
                    # Final tap: the four blocks' mult+add bunch behind the
                    # last weight bytes; block 0 runs on the idle GpSimd so
                    # DVE starts block 1 immediately.
                    eng = nc.gpsimd if (pos >= 47 and b == 0) else nc.vector
                    eng.tensor_tensor(
                        out=pq[:], in0=wbq[:], in1=src_for(t, b, 1)[:, 0, :], op=MULT
                    )
                    if last:
                        eng.tensor_tensor(
                            out=outf[:, b, :], in0=acc[:, b, :], in1=pq[:], op=ADD
                        )
                    else:
                        nc.vector.tensor_tensor(
                            out=acc[:, b, :], in0=acc[:, b, :], in1=pq[:], op=ADD
                        )
            # Store order follows readiness: DVE finishes block 1 first,
            # Pool's block 0 next, then blocks 2 and 3.
            for b in (1, 0, 2, 3):
                nc.sync.dma_start(out=out_r[:, b, :], in_=outf[:, b, :])

    nc.compile()
    return nc


def get_nc():
    if "nc" not in _CACHE:
        _CACHE["nc"] = _build_nc()
    return _CACHE["nc"]


def kernel(guide_weight, hn, h0):
    from concourse.bass_utils import run_bass_kernel_spmd

    nc = get_nc()
    in_maps = [
        {
            "gw": np.ascontiguousarray(guide_weight[b], dtype=np.float32),
            "hn": np.ascontiguousarray(hn[b, 0], dtype=np.float32),
            "h0": np.ascontiguousarray(h0[b, 0], dtype=np.float32),
        }
        for b in range(8)
    ]
    res = run_bass_kernel_spmd(nc, in_maps, core_ids=list(range(8)))
    return np.stack([res.results[b]["out"] for b in range(8)])[:, None].astype(
        np.float32
    )


# revision 27
# speedup vs baseline: 1.0004x; 1.0004x over previous
"""CSPN (7x7 per-pixel spatial propagation) Trainium2 kernel.

Problem: out[b,0,y,x] = sum_{i,j in 0..6} gw[b, 7i+j, y+3, x+3] * src(y+3-i, x+3-j)
where src = hn (zero-padded outside [0,512)) except the center tap (i=j=3)
which uses h0. Shapes: gw [8,49,518,518] f32, hn/h0 [8,1,512,512] f32.

Strategy: pure data parallel - one batch element per NeuronCore (8 cores).

Layout: image row r lives at partition r//4, row-block b = r%4 (each
partition holds 4 consecutive rows). A vertical shift of up to +-3 rows
then moves at most +-1 partition, and every shifted read window is a
plain strided slice of ONE halo tensor:

    s0[p, m, 3+c] = hn[4p + m - 3, c]   for m in [0, 10)

Tap (i, j) with dr = 3-i, dc = 3-j reads s0[:, dr+3 : dr+7, 3+dc : 515+dc].
The slice's element offset is even exactly when j is even; a second copy
s1 one slot to the right serves odd-j taps so bf16 DVE reads stay
4B-aligned (2x mode).

The halo is built with a single 1 MB DMA of hn: the mid blocks (m=3..6)
are cast from the f32 staging tile, and the +-1-partition-shifted blocks
(m=0..2, 7..9) are produced on the otherwise-idle PE array as matmuls
with sub/super-diagonal permutation matrices (built via affine_select),
whose all-zero edge columns also provide the top/bottom zero padding for
free. After startup the DMA engines stream nothing but the 49
guide-weight planes (51.4 MB/core - the memory-roofline term), h0, and
the final 1 MB output store; the modeled DMA bus never idles mid-stream.

Engine roles:
 - SP sequencer: pure DMA issue ring (staging, h0, every weight plane,
   output stores) - no compute waits can stall it.
 - ACT: all f32->bf16 weight casts (so DVE multiplies run in 2x mode).
 - DVE: per-tap multiply + accumulate chain (bf16).
 - GpSimd (Pool): takes six early taps on a second accumulator (merged
   once mid-stream) plus the first tail tap per block, keeping DVE
   slack so the final adds fire as soon as the last weight bytes land.
 - PE: the six halo shift matmuls.

Tap order runs the shift-free row (i=3) first and the i=0 row last, so
the PE-produced halo blocks are needed only well after they are ready.
Tail: the last taps shrink to half-tiles then per-row-block quarters;
the final tap's quarters multiply straight from f32 weights (no cast
hop) and the final add emits f32 into outf, which SP streams out.
"""

import numpy as np

_CACHE = {}

# Row i=3 (no vertical shift) first, i=0 (needs all up-blocks) last.
TAP_ORDER = [7 * i + j for i in (3, 4, 2, 5, 1, 6, 0) for j in range(7)]
POOL_POS = (1, 4, 7, 10, 13, 16)  # positions offloaded to GpSimd
MERGE_POS = 28  # position after which acc2 merges into acc (must be < 30)


def _build_nc():
    import concourse.bacc as bacc
    import concourse.mybir as mybir
    import concourse.tile as tile

    F32 = mybir.dt.float32
    BF16 = mybir.dt.bfloat16
    MULT = mybir.AluOpType.mult
    ADD = mybir.AluOpType.add
    EQ = mybir.AluOpType.is_equal

    nc = bacc.Bacc("TRN2", target_bir_lowering=False, debug=False, num_devices=8)
    gw = nc.dram_tensor("gw", [49, 518, 518], F32, kind="ExternalInput").ap()
    hn = nc.dram_tensor("hn", [512, 512], F32, kind="ExternalInput").ap()
    h0 = nc.dram_tensor("h0", [512, 512], F32, kind="ExternalInput").ap()
    out = nc.dram_tensor("out", [512, 512], F32, kind="ExternalOutput").ap()

    with tile.TileContext(nc) as tc:
        with (
            tc.tile_pool(name="persist", bufs=1) as pp,
            tc.tile_pool(name="wf", bufs=4) as wfp,
            tc.tile_pool(name="wb", bufs=4) as wbp,
            tc.tile_pool(name="wb2", bufs=2) as wb2p,
            tc.tile_pool(name="prod", bufs=2) as prp,
            tc.tile_pool(name="p2", bufs=2) as p2p,
            tc.tile_pool(name="wq12", bufs=12) as wqp,
            tc.tile_pool(name="whalf", bufs=4) as whp,
            tc.tile_pool(name="wsmall", bufs=6) as wsp,
            tc.tile_pool(name="prodq", bufs=4) as pqp,
            tc.tile_pool(name="ps", bufs=2, space="PSUM") as psp,
        ):
            # --- halo staging -------------------------------------------
            stage = pp.tile([128, 4, 512], F32, tag="stage")
            nc.sync.dma_start(out=stage[:], in_=hn.rearrange("(p b) x -> p b x", b=4))
            h0f = pp.tile([128, 4, 512], F32)
            nc.sync.dma_start(out=h0f[:], in_=h0.rearrange("(p b) x -> p b x", b=4))

            s0 = pp.tile([128, 10, 520], BF16, tag="s0")
            s1 = pp.tile([128, 10, 520], BF16, tag="s1")
            nc.vector.memset(s0[:, :, 0:3], 0.0)
            nc.vector.memset(s0[:, :, 515:520], 0.0)
            nc.vector.memset(s1[:, :, 0:4], 0.0)
            nc.vector.memset(s1[:, :, 516:520], 0.0)
            nc.scalar.copy(out=s0[:, 3:7, 3:515], in_=stage[:])
            nc.vector.tensor_copy(s1[:, 3:7, 4:516], s0[:, 3:7, 3:515])
            h0b = pp.tile([128, 4, 512], BF16)
            nc.scalar.copy(out=h0b[:], in_=h0f[:])

            # Shift matrices: Tup[q, p] = [q == p+1], Tdn[q, p] = [q == p-1].
            # As matmul lhsT they realize out[p] = in[p+-1]; their all-zero
            # first/last columns zero the out-of-image rows automatically.
            ones = pp.tile([128, 128], BF16, tag="ones")
            nc.gpsimd.memset(ones[:], 1.0)
            tup = pp.tile([128, 128], BF16, tag="tup")
            nc.gpsimd.affine_select(
                out=tup[:], in_=ones[:], pattern=[[-1, 128]], compare_op=EQ,
                fill=0.0, base=-1, channel_multiplier=1,
            )
            tdn = pp.tile([128, 128], BF16, tag="tdn")
            nc.gpsimd.affine_select(
                out=tdn[:], in_=ones[:], pattern=[[-1, 128]], compare_op=EQ,
                fill=0.0, base=1, channel_multiplier=1,
            )

            # Up blocks m=7+r hold row 4p+4+r = mid block r of partition p+1;
            # dn blocks m=r hold row 4p+r-3 = mid block r+1 of partition p-1.
            # rhs reads the 4B-aligned s1 mid copy. Emission order matches
            # first use: i=4 needs m=2 first, i=2 needs m=7, etc.
            for kind, r in (("dn", 2), ("up", 0), ("dn", 1), ("up", 1), ("dn", 0), ("up", 2)):
                ps = psp.tile([128, 512], F32, tag="ps")
                if kind == "up":
                    nc.tensor.matmul(ps[:], tup[:], s1[:, 3 + r, 4:516])
                    dst = 7 + r
                else:
                    nc.tensor.matmul(ps[:], tdn[:], s1[:, 4 + r, 4:516])
                    dst = r
                nc.scalar.copy(out=s0[:, dst, 3:515], in_=ps[:])
            nc.vector.tensor_copy(s1[:, 0:3, 4:516], s0[:, 0:3, 3:515])
            nc.vector.tensor_copy(s1[:, 7:10, 4:516], s0[:, 7:10, 3:515])

            # --- tap machinery ------------------------------------------
            acc = pp.tile([128, 4, 512], BF16)
            acc2 = pp.tile([128, 4, 512], BF16)
            outf = pp.tile([128, 4, 512], F32)
            out_r = out.rearrange("(p b) x -> p b x", b=4)
            gw_r = [
                gw[t, 3:515, 3:515].rearrange("(p b) x -> p b x", b=4)
                for t in range(49)
            ]

            def src_for(t, b0=0, nb=4):
                i, j = t // 7, t % 7
                if t == 24:
                    return h0b[:, b0 : b0 + nb, :]
                dr, dc = 3 - i, 3 - j
                if j % 2 == 0:
                    return s0[:, dr + 3 + b0 : dr + 3 + b0 + nb, 3 + dc : 515 + dc]
                return s1[:, dr + 3 + b0 : dr + 3 + b0 + nb, 4 + dc : 516 + dc]

            # Positions 0..43 stream full-tile; six of them accumulate on
            # GpSimd into acc2, merged into acc once after the halves.
            pool_started = False
            for pos in range(36):
                t = TAP_ORDER[pos]
                wf = wfp.tile([128, 4, 512], F32, tag="wf")
                nc.sync.dma_start(out=wf[:], in_=gw_r[t])
                if pos in POOL_POS:
                    wb = wb2p.tile([128, 4, 512], BF16, tag="wb2")
                    nc.scalar.copy(out=wb[:], in_=wf[:])
                    if not pool_started:
                        nc.gpsimd.tensor_tensor(
                            out=acc2[:], in0=wb[:], in1=src_for(t), op=MULT
                        )
                        pool_started = True
                    else:
                        p2 = p2p.tile([128, 4, 512], BF16, tag="p2")
                        nc.gpsimd.tensor_tensor(
                            out=p2[:], in0=wb[:], in1=src_for(t), op=MULT
                        )
                        nc.gpsimd.tensor_tensor(
                            out=acc2[:], in0=acc2[:], in1=p2[:], op=ADD
                        )
                else:
                    wb = wbp.tile([128, 4, 512], BF16, tag="wb")
                    nc.scalar.copy(out=wb[:], in_=wf[:])
                    if pos == 0:
                        nc.vector.tensor_tensor(
                            out=acc[:], in0=wb[:], in1=src_for(t), op=MULT
                        )
                    else:
                        prod = prp.tile([128, 4, 512], BF16, tag="prod")
                        nc.vector.tensor_tensor(
                            out=prod[:], in0=wb[:], in1=src_for(t), op=MULT
                        )
                        nc.vector.tensor_tensor(
                            out=acc[:], in0=acc[:], in1=prod[:], op=ADD
                        )
                if pos == MERGE_POS:
                    nc.vector.tensor_tensor(out=acc[:], in0=acc[:], in1=acc2[:], op=ADD)

            # --- tail ----------------------------------------------------
            # The kernel's end waits on the cast->mult->add chain behind the
            # LAST weight bytes, so taper the pieces as the stream drains:
            # positions 36..46 run as half-tiles and 47..48 as per-row-block
            # quarters. The final tap's add emits f32 straight into outf
            # (no output cast), and SP streams the four block stores out.
            for pos in (36, 37, 38, 39, 40, 41, 42, 43, 44, 45, 46):
                t = TAP_ORDER[pos]
                for h in (0, 1):
                    wfh = whp.tile([128, 2, 512], F32, tag="wfh")
                    nc.sync.dma_start(out=wfh[:], in_=gw_r[t][:, 2 * h : 2 * h + 2, :])
                    wbh = wsp.tile([128, 2, 512], BF16, tag="wbh")
                    nc.scalar.copy(out=wbh[:], in_=wfh[:])
                    prodh = prp.tile([128, 2, 512], BF16, tag="prodh")
                    nc.vector.tensor_tensor(
                        out=prodh[:], in0=wbh[:], in1=src_for(t, 2 * h, 2), op=MULT
                    )
                    nc.vector.tensor_tensor(
                        out=acc[:, 2 * h : 2 * h + 2, :],
                        in0=acc[:, 2 * h : 2 * h + 2, :],
                        in1=prodh[:],
                        op=ADD,
                    )
            for pos in (42, 43):
                t = TAP_ORDER[pos]
                for b in range(4):
                    wq = wqp.tile([128, 512], F32, tag="wq")
                    nc.sync.dma_start(out=wq[:], in_=gw_r[t][:, b, :])
                    wbq = wsp.tile([128, 512], BF16, tag="wbq")
                    nc.scalar.copy(out=wbq[:], in_=wq[:])
                    pq = pqp.tile([128, 512], BF16, tag="prodq")
                    nc.vector.tensor_tensor(
                        out=pq[:], in0=wbq[:], in1=src_for(t, b, 1)[:, 0, :], op=MULT
                    )
                    nc.vector.tensor_tensor(
                        out=acc[:, b, :], in0=acc[:, b, :], in1=pq[:], op=ADD
                    )
            # Taps at positions 44/45: DVE does only the multiplies; the
            # otherwise-idle GpSimd engine pair-adds the two products so DVE
            # spends one add instead of two per block. (z = p44 + p45 on
            # Pool; acc += z on DVE.)
            pr44, pr45 = [], []
            for pos, keep in ((44, pr44), (45, pr45)):
                t = TAP_ORDER[pos]
                for b in range(4):
                    wq = wqp.tile([128, 512], F32, tag="wq")
                    nc.sync.dma_start(out=wq[:], in_=gw_r[t][:, b, :])
                    wbq = wsp.tile([128, 512], BF16, tag="wbq")
                    nc.scalar.copy(out=wbq[:], in_=wq[:])
                    pq = tkp.tile([128, 512], BF16, tag=f"p{pos}")
                    nc.vector.tensor_tensor(
                        out=pq[:], in0=wbq[:], in1=src_for(t, b, 1)[:, 0, :], op=MULT
                    )
                    keep.append(pq)
            zs = []
            for b in range(4):
                z = tkp.tile([128, 512], BF16, tag="z1")
                nc.gpsimd.tensor_tensor(
                    out=z[:], in0=pr44[b][:], in1=pr45[b][:], op=ADD
                )
                zs.append(z)
            for b in range(4):
                nc.vector.tensor_tensor(
                    out=acc[:, b, :], in0=acc[:, b, :], in1=zs[b][:], op=ADD
                )
            for pos in (46, 47, 48):
                t = TAP_ORDER[pos]
                last = pos == 48
                for b in range(4):
                    wq = wqp.tile([128, 512], F32, tag="wq")
                    nc.sync.dma_start(out=wq[:], in_=gw_r[t][:, b, :])
                    wbq = wsp.tile([128, 512], BF16, tag="wbq")
                    nc.scalar.copy(out=wbq[:], in_=wq[:])
                    pq = pqp.tile([128, 512], BF16, tag="prodq")
                    # Final tap: the four blocks' mult+add bunch behind the
                    # last weight bytes; block 0 runs on the idle GpSimd so
                    # DVE starts block 1 immediately.
                    eng = nc.gpsimd if (last and b == 0) else nc.vector
                    eng.tensor_tensor(
                        out=pq[:], in0=wbq[:], in1=src_for(t, b, 1)[:, 0, :], op=MULT
                    )
                    if last:
                        eng.tensor_tensor(
                            out=outf[:, b, :], in0=acc[:, b, :], in1=pq[:], op=ADD
                        )
                    else:
                        nc.vector.tensor_tensor(
                            out=acc[:, b, :], in0=acc[:, b, :], in1=pq[:], op=ADD
                        )
            # Store order follows readiness: DVE finishes block 1 first,
            # Pool's block 0 next, then blocks 2 and 3.
            for b in (1, 0, 2, 3):
                nc.sync.dma_start(out=out_r[:, b, :], in_=outf[:, b, :])

    nc.compile()
    return nc


def get_nc():
    if "nc" not in _CACHE:
        _CACHE["nc"] = _build_nc()
    return _CACHE["nc"]


def kernel(guide_weight, hn, h0):
    from concourse.bass_utils import run_bass_kernel_spmd

    nc = get_nc()
    in_maps = [
        {
            "gw": np.ascontiguousarray(guide_weight[b], dtype=np.float32),
            "hn": np.ascontiguousarray(hn[b, 0], dtype=np.float32),
            "h0": np.ascontiguousarray(h0[b, 0], dtype=np.float32),
        }
        for b in range(8)
    ]
    res = run_bass_kernel_spmd(nc, in_maps, core_ids=list(range(8)))
    return np.stack([res.results[b]["out"] for b in range(8)])[:, None].astype(
        np.float32
    )
